# revision 49
# baseline (speedup 1.0000x reference)
"""AdditiveAttention Trainium2 kernel.

reference:
    q_proj  = query @ Wq.T                              (B,H)
    k_proj  = einsum('bsh,gh->bsg', keys, Wk)           (B,S,H)
    scores  = einsum('bsh,h->bs', tanh(q_proj[:,None,:]+k_proj), v)
    scores  = where(mask, -1e9, scores)
    weights = softmax(scores, axis=1)                   (B,S)
    context = einsum('bs,bsh->bh', weights, keys)       (B,H)
    return (context, weights)

Sharding: data-parallel over batch, 4 batches per core on 8 cores.
Wk/Wq/v replicated. Host does layout-only transforms (transposes /
reshapes); all FLOPs run on device.

Per-core device pipeline (B'=4 local batches), per (b, sc) chunk of
512 score columns:
  - keysT (h on partitions) resident in SBUF: (128, 16, 2048).
  - k_projT tile (g=128, s=512) = sum_ht WkT[ht,g].T @ keysT[ht,s]
    as fp32r matmuls (PSUM accumulation).
  - tanh fused with the q_proj bias on ScalarE -> combined in SBUF.
  - scores chunk (1,512): 4 M=1 v-matmuls + one rank-1 mask matmul
    (NEG_MASK * mask) accumulated in PSUM.
  - exp on ScalarE (accum_out = chunk sum); result chunk u at p0.
  - u broadcast across partitions with a ones(1,128) matmul (PSUM),
    then context partial sums via fused scalar_tensor_tensor on
    VectorE against keysT (contraction over s, no 2nd keys layout).
  - per batch: 1/sum scaling for both outputs.
"""

import sys

for _p in ("/opt/trn_rl_repo", "/root/.axon_site/_ro/trn_rl_repo"):
    if _p not in sys.path:
        sys.path.insert(0, _p)

import numpy as np
import ml_dtypes

import concourse.bass as bass
import concourse.bacc as bacc
import concourse.tile as tile
from concourse import mybir
from concourse import bass_utils

F32 = mybir.dt.float32
F32R = mybir.dt.float32r
BF16 = mybir.dt.bfloat16
U8 = mybir.dt.uint8
AF = mybir.ActivationFunctionType
ALU = mybir.AluOpType
AX = mybir.AxisListType

B, S, H = 32, 2048, 512
N_CORES = 8
BPC = B // N_CORES  # batches per core

NEG_MASK = -60.0  # added to masked scores: exp(s-60) ~ 1e-26 * exp(s)


def build_program(bpc=BPC, s=S, h=H, kproj_dtype=F32R):
    """Build + compile the per-core Bass program (SPMD, same on all cores)."""
    nc = bacc.Bacc("TRN2", target_bir_lowering=False, debug=False)
    ht_n = h // 128   # tiles along contraction h
    gt_n = h // 128   # tiles along output g
    sc_n = s // 512   # 512-wide score chunks
    sc_w = 512

    kd = kproj_dtype
    keysT_d = nc.dram_tensor("keysT", (bpc, h, s), kd, kind="ExternalInput")
    qT_d = nc.dram_tensor("queryT", (h, bpc), F32, kind="ExternalInput")
    mask_d = nc.dram_tensor("mask_bf16", (1, bpc * s), BF16, kind="ExternalInput")
    wkT_d = nc.dram_tensor("wkT", (h, h), kd, kind="ExternalInput")
    wqT_d = nc.dram_tensor("wqT", (h, h), F32, kind="ExternalInput")
    v_d = nc.dram_tensor("v4", (128, gt_n), kd, kind="ExternalInput")
    ctx_d = nc.dram_tensor("ctx", (bpc, h), F32, kind="ExternalOutput")
    w_d = nc.dram_tensor("w", (bpc, s), F32, kind="ExternalOutput")

    with tile.TileContext(nc) as tc:
        with (
            tc.tile_pool(name="consts", bufs=1) as consts,
            tc.tile_pool(name="keysp", bufs=1) as keysp,
            tc.tile_pool(name="work", bufs=3) as work,
            tc.tile_pool(name="combp", bufs=4) as combp,
            tc.tile_pool(name="psum", bufs=3, space="PSUM") as psum,
        ):
            # ---- small consts first so q_proj can start immediately ----
            wk_sb = consts.tile([128, ht_n, h], kd)
            wkT_r = wkT_d.ap().rearrange("(ht p) g -> p ht g", p=128)
            for gt in range(gt_n):
                nc.sync.dma_start(wk_sb[:, :, bass.ds(gt * 128, 128)],
                                  wkT_r[:, :, bass.ds(gt * 128, 128)])
            wq_sb = work.tile([128, ht_n, h], F32, tag="wq", bufs=1, name="wq_sb")
            nc.sync.dma_start(wq_sb[:], wqT_d.ap().rearrange("(ht p) g -> p ht g", p=128))
            qT_sb = consts.tile([128, ht_n, bpc], F32)
            nc.sync.dma_start(qT_sb[:], qT_d.ap().rearrange("(ht p) c -> p ht c", p=128))
            v_sb = consts.tile([128, gt_n], kd)
            nc.sync.dma_start(v_sb[:], v_d.ap())
            mask_sb = consts.tile([1, bpc * s], BF16)
            nc.sync.dma_start(mask_sb[:], mask_d.ap())

            neg1_f = consts.tile([1, 1], F32)
            nc.vector.memset(neg1_f[:], NEG_MASK)
            neg1_sb = consts.tile([1, 1], BF16)
            nc.vector.tensor_copy(neg1_sb[:], neg1_f[:])
            ones_f = consts.tile([1, 128], F32)
            nc.vector.memset(ones_f[:], 1.0)
            ones_sb = consts.tile([1, 128], kd)
            nc.vector.tensor_copy(ones_sb[:], ones_f[:])

            # ---- keys: batch 0 split finely across queues, rest whole ----
            keys_sb = keysp.tile([128, bpc * ht_n, s], kd)
            keysT_r = keysT_d.ap().rearrange("b (ht p) s -> p (b ht) s", p=128)
            for half in range(2):
                for ht in range(ht_n):
                    nc.sync.dma_start(
                        keys_sb[:, ht, bass.ds(half * (s // 2), s // 2)],
                        keysT_r[:, ht, bass.ds(half * (s // 2), s // 2)],
                    )
            for b in range(1, bpc):
                for ht in range(ht_n):
                    i = b * ht_n + ht
                    for q in range(2):
                        nc.sync.dma_start(
                            keys_sb[:, i, bass.ds(q * (s // 2), s // 2)],
                            keysT_r[:, i, bass.ds(q * (s // 2), s // 2)],
                        )

            # ---- q_proj (exact fp32; tiny) ----
            qp_sb = consts.tile([128, gt_n * bpc], F32)
            for gt in range(gt_n):
                qp_ps = psum.tile([128, bpc], F32, tag="kp", bufs=3)
                for ht in range(ht_n):
                    nc.tensor.matmul(
                        qp_ps[:],
                        wq_sb[:, ht, gt * 128:(gt + 1) * 128],
                        qT_sb[:, ht, :],
                        start=(ht == 0),
                        stop=(ht == ht_n - 1),
                    )
                nc.vector.tensor_copy(qp_sb[:, gt * bpc:(gt + 1) * bpc], qp_ps[:])

            # ---- bookkeeping ----
            sums_sb = consts.tile([1, bpc * sc_n], F32)

            u_sb = work.tile([128, s], F32, tag="u_sb", bufs=1, name="u_sb")

            # Work that must trail the PE stream by one step so the PE
            # never waits on ScalarE: a FIFO popped once per tanh.
            pending = []
            acc_all = [None] * bpc

            def flush_pending():
                while pending:
                    pending.pop(0)()

            def ctx_tail(b, sc, sc_ps):
                """exp + partition-broadcast + context partials for a chunk;
                per-batch finalization on the last chunk."""
                stage = work.tile([1, sc_w], kd, tag="stage", bufs=3)
                nc.scalar.activation(
                    stage[:], sc_ps[:], AF.Exp,
                    accum_out=sums_sb[0:1, b * sc_n + sc: b * sc_n + sc + 1],
                )
                if kd == BF16:
                    nc.gpsimd.dma_start(
                        u_sb[32 * b:32 * b + 1, bass.ds(sc * sc_w, sc_w)],
                        stage[:],
                    )
                else:
                    nc.sync.dma_start(
                        u_sb[32 * b:32 * b + 1, bass.ds(sc * sc_w, sc_w)],
                        stage[:].bitcast(F32),
                    )

                def bcast_and_stt(b=b, sc=sc, stage=stage):
                    urep_ps = psum.tile([128, sc_w], F32, tag="urep", bufs=2,
                                        name="urep_ps")
                    nc.tensor.matmul(urep_ps[:], ones_sb[:], stage[:],
                                     start=True, stop=True)
                    if sc == 0:
                        acc_all[b] = work.tile(
                            [128, ht_n, sc_n], F32, tag="accall", bufs=2,
                            name="acc_all",
                        )
                    for ht in range(ht_n):
                        junk = work.tile([128, sc_w], F32, tag="junk", bufs=2)
                        nc.vector.scalar_tensor_tensor(
                            out=junk[:],
                            in0=(lambda ap: ap.bitcast(F32) if kd == F32R
                                 else ap)(keys_sb[:, b * ht_n + ht,
                                          bass.ds(sc * sc_w, sc_w)]),
                            scalar=1.0,
                            in1=urep_ps[:],
                            op0=ALU.mult,
                            op1=ALU.mult,
                            accum_out=acc_all[b][:, ht, sc:sc + 1],
                        )
                    if sc == sc_n - 1:
                        bsum = work.tile([1, 1], F32, tag="bsum", bufs=4)
                        nc.vector.reduce_sum(
                            bsum[:], sums_sb[0:1, b * sc_n:(b + 1) * sc_n],
                            axis=AX.X,
                        )
                        srep_ps = psum.tile([128, 1], F32, tag="urep", bufs=2,
                                            name="srep_ps")
                        nc.tensor.matmul(srep_ps[:], ones_f[:], bsum[:],
                                         start=True, stop=True)
                        rrep = work.tile([128, 1], F32, tag="rrep", bufs=4)
                        nc.vector.reciprocal(rrep[:], srep_ps[:])
                        acc4 = work.tile([128, ht_n], F32, tag="acc4", bufs=4)
                        nc.vector.reduce_sum(acc4[:], acc_all[b][:], axis=AX.X)
                        ctxc = work.tile([128, ht_n], F32, tag="ctxc", bufs=4)
                        nc.vector.tensor_scalar_mul(ctxc[:], acc4[:], rrep[:])
                        dst = bass.AP(
                            tensor=ctx_d.ap().tensor,
                            offset=b * h,
                            ap=[[1, 128], [128, ht_n]],
                        )
                        nc.sync.dma_start(dst, ctxc[:])
                        # weights row: u * (1/sum) using rrep's partition-b
                        # copy of the same scalar, then stream the row out
                        nc.scalar.mul(
                            u_sb[32 * b:32 * b + 1, :],
                            u_sb[32 * b:32 * b + 1, :],
                            rrep[32 * b:32 * b + 1, :],
                        )
                        nc.sync.dma_start(w_d.ap()[b:b + 1, :],
                                          u_sb[32 * b:32 * b + 1, :])

                pending.append(bcast_and_stt)

            for b in range(bpc):
                for sc in range(sc_n):
                    ssl = bass.ds(sc * sc_w, sc_w)
                    sc_ps = psum.tile([1, sc_w], F32, tag="sc", bufs=3)
                    for gt in range(gt_n):
                        kp_ps = psum.tile([128, sc_w], F32, tag="kp", bufs=3)
                        for ht in range(ht_n):
                            nc.tensor.matmul(
                                kp_ps[:],
                                wk_sb[:, ht, gt * 128:(gt + 1) * 128],
                                keys_sb[:, b * ht_n + ht, ssl],
                                start=(ht == 0),
                                stop=(ht == ht_n - 1),
                            )
                        comb = combp.tile([128, sc_w], kd, tag="comb", bufs=5)
                        nc.scalar.activation(
                            comb[:], kp_ps[:], AF.Tanh,
                            bias=qp_sb[:, gt * bpc + b: gt * bpc + b + 1],
                        )
                        if pending:
                            pending.pop(0)()

                        def score_mm(sc_ps=sc_ps, comb=comb, gt=gt, b=b,
                                     sc=sc):
                            nc.tensor.matmul(
                                sc_ps[:],
                                v_sb[:, gt:gt + 1],
                                comb[:],
                                start=(gt == 0),
                                stop=False,
                            )
                            if gt == gt_n - 1:
                                # mask as one more rank-1 update:
                                # scores += NEG_MASK * mask
                                nc.tensor.matmul(
                                    sc_ps[:],
                                    neg1_sb[:],
                                    mask_sb[0:1, b * s + sc * sc_w:
                                            b * s + (sc + 1) * sc_w],
                                    start=False,
                                    stop=True,
                                )
                                ctx_tail(b=b, sc=sc, sc_ps=sc_ps)
                        pending.append(score_mm)

            flush_pending()



    nc.compile()
    return nc


_NC_CACHE = {}


def get_program(**kw):
    key = tuple(sorted(kw.items()))
    if key not in _NC_CACHE:
        _NC_CACHE[key] = build_program(**kw)
    return _NC_CACHE[key]


def make_in_maps(query, keys, mask, Wq, Wk, v, n_cores=N_CORES,
                 kdtype=np.float32):
    """Host-side sharding + layout-only transforms (no math)."""
    query = np.asarray(query, dtype=np.float32)
    keys = np.asarray(keys, dtype=np.float32).astype(kdtype)
    mask = np.asarray(mask)
    wkT = np.ascontiguousarray(np.asarray(Wk, np.float32).T.astype(kdtype))
    wqT = np.ascontiguousarray(np.asarray(Wq, np.float32).T)
    v4 = np.ascontiguousarray(np.asarray(v, np.float32).reshape(-1, 128).T.astype(kdtype))
    bpc = query.shape[0] // n_cores
    in_maps = []
    for c in range(n_cores):
        sl = slice(c * bpc, (c + 1) * bpc)
        in_maps.append({
            "keysT": np.ascontiguousarray(keys[sl].transpose(0, 2, 1)),
            "queryT": np.ascontiguousarray(query[sl].T),
            "mask_bf16": np.ascontiguousarray(mask[sl]).reshape(1, -1).astype(ml_dtypes.bfloat16),
            "wkT": wkT,
            "wqT": wqT,
            "v4": v4,
        })
    return in_maps


def kernel(query, keys, mask, Wq, Wk, v):
    nc = get_program()
    in_maps = make_in_maps(query, keys, mask, Wq, Wk, v)
    last_err = None
    for _attempt in range(3):
        try:
            res = bass_utils.run_bass_kernel_spmd(
                nc, in_maps, core_ids=list(range(N_CORES))
            )
            break
        except Exception as e:  # transient device wedges recover on retry
            last_err = e
            res = None
    if res is None:
        raise last_err
    context = np.concatenate([res.results[c]["ctx"] for c in range(N_CORES)], axis=0)
    weights = np.concatenate([res.results[c]["w"] for c in range(N_CORES)], axis=0)
    return context, weights


# revision 50
# speedup vs baseline: 1.0787x; 1.0787x over previous
"""AdditiveAttention Trainium2 kernel.

reference:
    q_proj  = query @ Wq.T                              (B,H)
    k_proj  = einsum('bsh,gh->bsg', keys, Wk)           (B,S,H)
    scores  = einsum('bsh,h->bs', tanh(q_proj[:,None,:]+k_proj), v)
    scores  = where(mask, -1e9, scores)
    weights = softmax(scores, axis=1)                   (B,S)
    context = einsum('bs,bsh->bh', weights, keys)       (B,H)
    return (context, weights)

Sharding: data-parallel over batch, 4 batches per core on 8 cores.
Wk/Wq/v replicated. Host does layout-only transforms (transposes /
reshapes); all FLOPs run on device.

Per-core device pipeline (B'=4 local batches), per (b, sc) chunk of
512 score columns:
  - keysT (h on partitions) resident in SBUF: (128, 16, 2048).
  - k_projT tile (g=128, s=512) = sum_ht WkT[ht,g].T @ keysT[ht,s]
    as fp32r matmuls (PSUM accumulation).
  - tanh fused with the q_proj bias on ScalarE -> combined in SBUF.
  - scores chunk (1,512): 4 M=1 v-matmuls + one rank-1 mask matmul
    (NEG_MASK * mask) accumulated in PSUM.
  - exp on ScalarE (accum_out = chunk sum); result chunk u at p0.
  - u broadcast across partitions with a ones(1,128) matmul (PSUM),
    then context partial sums via fused scalar_tensor_tensor on
    VectorE against keysT (contraction over s, no 2nd keys layout).
  - per batch: 1/sum scaling for both outputs.
"""

import sys

for _p in ("/opt/trn_rl_repo", "/root/.axon_site/_ro/trn_rl_repo"):
    if _p not in sys.path:
        sys.path.insert(0, _p)

import numpy as np
import ml_dtypes

import concourse.bass as bass
import concourse.bacc as bacc
import concourse.tile as tile
from concourse import mybir
from concourse import bass_utils

F32 = mybir.dt.float32
F32R = mybir.dt.float32r
BF16 = mybir.dt.bfloat16
U8 = mybir.dt.uint8
AF = mybir.ActivationFunctionType
ALU = mybir.AluOpType
AX = mybir.AxisListType

B, S, H = 32, 2048, 512
N_CORES = 8
BPC = B // N_CORES  # batches per core

NEG_MASK = -60.0  # added to masked scores: exp(s-60) ~ 1e-26 * exp(s)


def build_program(bpc=BPC, s=S, h=H, kproj_dtype=F32R):
    """Build + compile the per-core Bass program (SPMD, same on all cores)."""
    nc = bacc.Bacc("TRN2", target_bir_lowering=False, debug=False)
    ht_n = h // 128   # tiles along contraction h
    gt_n = h // 128   # tiles along output g
    sc_n = s // 512   # 512-wide score chunks
    sc_w = 512

    kd = kproj_dtype
    keysT_d = nc.dram_tensor("keysT", (bpc, h, s), kd, kind="ExternalInput")
    qT_d = nc.dram_tensor("queryT", (h, bpc), F32, kind="ExternalInput")
    mask_d = nc.dram_tensor("mask_bf16", (1, bpc * s), BF16, kind="ExternalInput")
    wkT_d = nc.dram_tensor("wkT", (h, h), kd, kind="ExternalInput")
    wqT_d = nc.dram_tensor("wqT", (h, h), F32, kind="ExternalInput")
    v_d = nc.dram_tensor("v4", (128, gt_n), kd, kind="ExternalInput")
    ctx_d = nc.dram_tensor("ctx", (bpc, h), F32, kind="ExternalOutput")
    w_d = nc.dram_tensor("w", (bpc, s), F32, kind="ExternalOutput")

    with tile.TileContext(nc) as tc:
        with (
            tc.tile_pool(name="consts", bufs=1) as consts,
            tc.tile_pool(name="keysp", bufs=1) as keysp,
            tc.tile_pool(name="work", bufs=3) as work,
            tc.tile_pool(name="combp", bufs=4) as combp,
            tc.tile_pool(name="psum", bufs=3, space="PSUM") as psum,
        ):
            # ---- small consts first so q_proj can start immediately ----
            wk_sb = consts.tile([128, ht_n, h], kd)
            wkT_r = wkT_d.ap().rearrange("(ht p) g -> p ht g", p=128)
            for gt in range(gt_n):
                nc.sync.dma_start(wk_sb[:, :, bass.ds(gt * 128, 128)],
                                  wkT_r[:, :, bass.ds(gt * 128, 128)])
            wq_sb = work.tile([128, ht_n, h], F32, tag="wq", bufs=1, name="wq_sb")
            nc.sync.dma_start(wq_sb[:], wqT_d.ap().rearrange("(ht p) g -> p ht g", p=128))
            qT_sb = consts.tile([128, ht_n, bpc], F32)
            nc.sync.dma_start(qT_sb[:], qT_d.ap().rearrange("(ht p) c -> p ht c", p=128))
            v_sb = consts.tile([128, gt_n], kd)
            nc.sync.dma_start(v_sb[:], v_d.ap())
            mask_sb = consts.tile([1, bpc * s], BF16)
            nc.sync.dma_start(mask_sb[:], mask_d.ap())

            neg1_f = consts.tile([1, 1], F32)
            nc.vector.memset(neg1_f[:], NEG_MASK)
            neg1_sb = consts.tile([1, 1], BF16)
            nc.vector.tensor_copy(neg1_sb[:], neg1_f[:])
            ones_f = consts.tile([1, 128], F32)
            nc.vector.memset(ones_f[:], 1.0)
            ones_sb = consts.tile([1, 128], kd)
            nc.vector.tensor_copy(ones_sb[:], ones_f[:])

            # ---- keys: batch 0 split finely across queues, rest whole ----
            keys_sb = keysp.tile([128, bpc * ht_n, s], kd)
            keysT_r = keysT_d.ap().rearrange("b (ht p) s -> p (b ht) s", p=128)
            for half in range(2):
                for ht in range(ht_n):
                    nc.sync.dma_start(
                        keys_sb[:, ht, bass.ds(half * (s // 2), s // 2)],
                        keysT_r[:, ht, bass.ds(half * (s // 2), s // 2)],
                    )
            for b in range(1, bpc):
                for ht in range(ht_n):
                    i = b * ht_n + ht
                    for q in range(2):
                        nc.sync.dma_start(
                            keys_sb[:, i, bass.ds(q * (s // 2), s // 2)],
                            keysT_r[:, i, bass.ds(q * (s // 2), s // 2)],
                        )

            # ---- q_proj (exact fp32; tiny) ----
            qp_sb = consts.tile([128, gt_n * bpc], F32)
            for gt in range(gt_n):
                qp_ps = psum.tile([128, bpc], F32, tag="kp", bufs=3)
                for ht in range(ht_n):
                    nc.tensor.matmul(
                        qp_ps[:],
                        wq_sb[:, ht, gt * 128:(gt + 1) * 128],
                        qT_sb[:, ht, :],
                        start=(ht == 0),
                        stop=(ht == ht_n - 1),
                    )
                nc.vector.tensor_copy(qp_sb[:, gt * bpc:(gt + 1) * bpc], qp_ps[:])

            # ---- bookkeeping ----
            sums_sb = consts.tile([1, bpc * sc_n], F32)

            u_sb = work.tile([128, s], F32, tag="u_sb", bufs=1, name="u_sb")

            # Work that must trail the PE stream by one step so the PE
            # never waits on ScalarE: a FIFO popped once per tanh.
            pending = []
            acc_all = [None] * bpc

            def flush_pending():
                while pending:
                    pending.pop(0)()

            def ctx_tail(b, sc, sc_ps):
                """exp + partition-broadcast + context partials for a chunk;
                per-batch finalization on the last chunk."""
                stage = work.tile([1, sc_w], kd, tag="stage", bufs=3)
                nc.scalar.activation(
                    stage[:], sc_ps[:], AF.Exp,
                    accum_out=sums_sb[0:1, b * sc_n + sc: b * sc_n + sc + 1],
                )
                if kd == BF16:
                    nc.gpsimd.dma_start(
                        u_sb[32 * b:32 * b + 1, bass.ds(sc * sc_w, sc_w)],
                        stage[:],
                    )
                else:
                    nc.sync.dma_start(
                        u_sb[32 * b:32 * b + 1, bass.ds(sc * sc_w, sc_w)],
                        stage[:].bitcast(F32),
                    )

                def bcast_and_stt(b=b, sc=sc, stage=stage):
                    urep_ps = psum.tile([128, sc_w], F32, tag="urep", bufs=2,
                                        name="urep_ps")
                    nc.tensor.matmul(urep_ps[:], ones_sb[:], stage[:],
                                     start=True, stop=True)
                    if sc == 0:
                        acc_all[b] = work.tile(
                            [128, ht_n, sc_n], F32, tag="accall", bufs=2,
                            name="acc_all",
                        )
                    for ht in range(ht_n):
                        junk = work.tile([128, sc_w], F32, tag="junk", bufs=2)
                        nc.vector.scalar_tensor_tensor(
                            out=junk[:],
                            in0=(lambda ap: ap.bitcast(F32) if kd == F32R
                                 else ap)(keys_sb[:, b * ht_n + ht,
                                          bass.ds(sc * sc_w, sc_w)]),
                            scalar=1.0,
                            in1=urep_ps[:],
                            op0=ALU.mult,
                            op1=ALU.mult,
                            accum_out=acc_all[b][:, ht, sc:sc + 1],
                        )
                    if sc == sc_n - 1:
                        bsum = work.tile([1, 1], F32, tag="bsum", bufs=4)
                        nc.vector.reduce_sum(
                            bsum[:], sums_sb[0:1, b * sc_n:(b + 1) * sc_n],
                            axis=AX.X,
                        )
                        srep_ps = psum.tile([128, 1], F32, tag="urep", bufs=2,
                                            name="srep_ps")
                        nc.tensor.matmul(srep_ps[:], ones_f[:], bsum[:],
                                         start=True, stop=True)
                        rrep = work.tile([128, 1], F32, tag="rrep", bufs=4)
                        nc.vector.reciprocal(rrep[:], srep_ps[:])
                        acc4 = work.tile([128, ht_n], F32, tag="acc4", bufs=4)
                        nc.vector.reduce_sum(acc4[:], acc_all[b][:], axis=AX.X)
                        ctxc = work.tile([128, ht_n], F32, tag="ctxc", bufs=4)
                        nc.vector.tensor_scalar_mul(ctxc[:], acc4[:], rrep[:])
                        dst = bass.AP(
                            tensor=ctx_d.ap().tensor,
                            offset=b * h,
                            ap=[[1, 128], [128, ht_n]],
                        )
                        nc.sync.dma_start(dst, ctxc[:])
                        # weights row: u * (1/sum) using rrep's partition-b
                        # copy of the same scalar, then stream the row out
                        nc.scalar.mul(
                            u_sb[32 * b:32 * b + 1, :],
                            u_sb[32 * b:32 * b + 1, :],
                            rrep[32 * b:32 * b + 1, :],
                        )
                        nc.sync.dma_start(w_d.ap()[b:b + 1, :],
                                          u_sb[32 * b:32 * b + 1, :])

                pending.append(bcast_and_stt)

            for b in range(bpc):
                for sc in range(sc_n):
                    ssl = bass.ds(sc * sc_w, sc_w)
                    sc_ps = psum.tile([1, sc_w], F32, tag="sc", bufs=3)
                    for gt in range(gt_n):
                        kp_ps = psum.tile([128, sc_w], F32, tag="kp", bufs=3)
                        for ht in range(ht_n):
                            nc.tensor.matmul(
                                kp_ps[:],
                                wk_sb[:, ht, gt * 128:(gt + 1) * 128],
                                keys_sb[:, b * ht_n + ht, ssl],
                                start=(ht == 0),
                                stop=(ht == ht_n - 1),
                            )
                        comb = combp.tile([128, sc_w], kd, tag="comb", bufs=5)
                        nc.scalar.activation(
                            comb[:], kp_ps[:], AF.Tanh,
                            bias=qp_sb[:, gt * bpc + b: gt * bpc + b + 1],
                        )
                        if pending:
                            pending.pop(0)()
                        if b == bpc - 1 and pending:
                            # last batch: drain eagerly so the final exps
                            # enter the ScalarE queue ahead of later tanhs
                            pending.pop(0)()

                        def score_mm(sc_ps=sc_ps, comb=comb, gt=gt, b=b,
                                     sc=sc):
                            nc.tensor.matmul(
                                sc_ps[:],
                                v_sb[:, gt:gt + 1],
                                comb[:],
                                start=(gt == 0),
                                stop=False,
                            )
                            if gt == gt_n - 1:
                                # mask as one more rank-1 update:
                                # scores += NEG_MASK * mask
                                nc.tensor.matmul(
                                    sc_ps[:],
                                    neg1_sb[:],
                                    mask_sb[0:1, b * s + sc * sc_w:
                                            b * s + (sc + 1) * sc_w],
                                    start=False,
                                    stop=True,
                                )
                                ctx_tail(b=b, sc=sc, sc_ps=sc_ps)
                        pending.append(score_mm)

            flush_pending()



    nc.compile()
    return nc


_NC_CACHE = {}


def get_program(**kw):
    key = tuple(sorted(kw.items()))
    if key not in _NC_CACHE:
        _NC_CACHE[key] = build_program(**kw)
    return _NC_CACHE[key]


def make_in_maps(query, keys, mask, Wq, Wk, v, n_cores=N_CORES,
                 kdtype=np.float32):
    """Host-side sharding + layout-only transforms (no math)."""
    query = np.asarray(query, dtype=np.float32)
    keys = np.asarray(keys, dtype=np.float32).astype(kdtype)
    mask = np.asarray(mask)
    wkT = np.ascontiguousarray(np.asarray(Wk, np.float32).T.astype(kdtype))
    wqT = np.ascontiguousarray(np.asarray(Wq, np.float32).T)
    v4 = np.ascontiguousarray(np.asarray(v, np.float32).reshape(-1, 128).T.astype(kdtype))
    bpc = query.shape[0] // n_cores
    in_maps = []
    for c in range(n_cores):
        sl = slice(c * bpc, (c + 1) * bpc)
        in_maps.append({
            "keysT": np.ascontiguousarray(keys[sl].transpose(0, 2, 1)),
            "queryT": np.ascontiguousarray(query[sl].T),
            "mask_bf16": np.ascontiguousarray(mask[sl]).reshape(1, -1).astype(ml_dtypes.bfloat16),
            "wkT": wkT,
            "wqT": wqT,
            "v4": v4,
        })
    return in_maps


def kernel(query, keys, mask, Wq, Wk, v):
    nc = get_program()
    in_maps = make_in_maps(query, keys, mask, Wq, Wk, v)
    last_err = None
    for _attempt in range(3):
        try:
            res = bass_utils.run_bass_kernel_spmd(
                nc, in_maps, core_ids=list(range(N_CORES))
            )
            break
        except Exception as e:  # transient device wedges recover on retry
            last_err = e
            res = None
    if res is None:
        raise last_err
    context = np.concatenate([res.results[c]["ctx"] for c in range(N_CORES)], axis=0)
    weights = np.concatenate([res.results[c]["w"] for c in range(N_CORES)], axis=0)
    return context, weights
